# revision 80
# baseline (speedup 1.0000x reference)
"""Trainium2 Bass kernel V3 for the PixelRNN Diagonal BiLSTM problem.

Contract: kernel(**inputs) takes FULL unsharded inputs and returns the FULL
(32, 3, 256, 32, 32) float32 output. Pure data-parallel over 8 NeuronCores
(4 images each), weights replicated, no collectives.

V3 vs V2 (see sim_v3.py for geometry validation):
  * Halved scan: only ~half the skewed-grid cells are real pixels. Prefix
    pad-cell LSTM states are input-independent (x=0 there), so the host
    precomputes the 17 possible pad states from the weights and each
    diagonal's scan starts at its first real cell with a per-diagonal init
    (h_init, c_init) table. Real segments are <= 16 steps, so the scan is
    16 macro-steps/dir over 4096 columns (vs 32 steps over 8064).
  * No scatter phase: the host packs the RAW pixels in scan-slot order
    (xT2 col = 64d + 16bi + s, bwd copy right-aligned within each 16-slot
    block), so the in_proj psum evictions write X/Xr directly with
    dst-contiguous 64-col runs. X layout col = 128d + 16bi + s leaves a
    dead upper half per diagonal; windows are offset 257s (fwd) and
    255s+15 (bwd) with strides [[128, nd], [16, 4]], nd = 94-4s.
  * un-scatters moved to the idle GPSIMD engine (3 pieces per dir-step).
  * out_proj: evictions go to per-m contiguous 4096-col slices (ACT for
    even m on dead X, DVE for odd m on dead Xr) and each m DMAs out as
    128 contiguous 8KB descriptors right after its eviction (the V2
    layout produced 4KB descriptors and a ~43 us DMA tail).
  * Cell state C is bf16 (DVE 2x perf mode; rel err 5.0e-3 vs 4.6e-3).
  * Scan matmuls run in the ACT consumer gate order (f,i,g,o) with
    per-gate psum-free pre-observers; ACT gate ops are dir-interleaved.
  * Input DMAs split across the SP and ACT HWDGE channels in need order;
    per-slice xT2 observers sit just before the first tile needing them.

Steady-state (HW, 8 cores): ~167 us vs the 285 us V2 baseline. The scan
(~106 us) is bound by the ACT engine's 10 transcendental ops per step
(~450 ns each incl. 185 ns fixed access cost); late steps reach 100% ACT
busy with the PE at the full 2.4 GHz p-state.

Per-core SBUF: X/Xr [128, 12288] bf16, hstall [128, 34, 384] bf16,
C [128, 2, 384] bf16, un [128, 8192] bf16, xT2 [3, 12288] bf16,
weights [128, 3584] bf16.
"""
from contextlib import ExitStack

import numpy as np

import concourse.bass as bass
import concourse.tile as tile
from concourse.tile import add_dep_helper
from concourse import mybir
from concourse.bass_utils import run_bass_kernel_spmd

AF = mybir.ActivationFunctionType
F32 = mybir.dt.float32
BF16 = mybir.dt.bfloat16
U32 = mybir.dt.uint32

BS = 4            # batch shard per core
NCORES = 8
H = W = 32
HC = 128
D = 94            # anti-diagonals of the skewed grid
S = 16            # max real-segment length
NCOLX = 96 * 128  # X/Xr alloc: 94 d-blocks (live) padded to 96


def _geom():
    d = np.arange(D)
    r0 = np.maximum(0, d - 62)
    r1 = np.minimum(31, d)
    a = np.maximum(r0, -(-(d - 31) // 2))
    b = np.minimum(r1, d // 2)
    L = b - a + 1
    return a, b, L, a - r0, r1 - b


def _ap(t, off, dims):
    """Free-dim-strided AP on SBUF tile t: dims = [[stride, count], ...]."""
    a = t[:, :]
    return bass.AP(tensor=a.tensor, offset=a.offset + off, ap=[a.ap[0]] + dims)


def build(nc):
    # ---------------- DRAM I/O ----------------
    xT2_d = nc.dram_tensor("xT2", (3, 2 * 6144), BF16, kind="ExternalInput")
    ipw_d = nc.dram_tensor("in_projT", (3, HC), BF16, kind="ExternalInput")
    wpack_d = nc.dram_tensor("wpack", (HC, 3584), BF16, kind="ExternalInput")
    vpack_d = nc.dram_tensor("vpack", (HC, 15), F32, kind="ExternalInput")
    hinit_d = nc.dram_tensor("hinit", (HC, 2 * 376), BF16, kind="ExternalInput")
    brow_d = nc.dram_tensor("brow", (1, 1152), BF16, kind="ExternalInput")
    cinit_d = nc.dram_tensor("cinit", (HC, 2 * 376), BF16,
                             kind="ExternalInput")
    out_d = nc.dram_tensor("out", (HC, 6, BS * 1024), BF16,
                           kind="ExternalOutput")

    with tile.TileContext(nc) as tc, ExitStack() as ctx:
        const = ctx.enter_context(tc.tile_pool(name="const", bufs=1))
        big = ctx.enter_context(tc.tile_pool(name="big", bufs=1))
        etmp = ctx.enter_context(tc.tile_pool(name="etmp", bufs=4))
        psum = ctx.enter_context(tc.tile_pool(name="psum", bufs=1,
                                              space="PSUM"))

        # DMA plan: each issuer's HWDGE channel moves its transfers serially,
        # so the SP carries the in_proj-critical path (ipw, xa1, vpk, xb1 in
        # need order) while ACT carries the rest in need order.
        final_insts = []
        ipw = const.tile([3, HC], BF16)
        xT2 = const.tile([3, 2 * 6144], BF16)
        vpk = const.tile([HC, 15], F32)
        wpk = const.tile([HC, 3584], BF16)
        cini = const.tile([HC, 2, 384], BF16, name="cini")
        C = const.tile([HC, 2, 384], BF16, name="C")
        # hstall: slice 0/1 = fwd/bwd init h; slice 2+2s+di = step-(s,dir) h
        hstall = const.tile([HC, 34, 384], BF16, name="hstall")

        brow = const.tile([1, 1152], BF16)
        final_insts.append(nc.sync.dma_start(ipw, ipw_d.ap()))
        final_insts.append(nc.sync.dma_start(brow, brow_d.ap()))
        final_insts.append(nc.sync.dma_start(xT2[:, 0:3072],
                                             xT2_d.ap()[:, 0:3072]))
        final_insts.append(nc.sync.dma_start(vpk, vpack_d.ap()))
        final_insts.append(nc.sync.dma_start(xT2[:, 6144:9216],
                                             xT2_d.ap()[:, 6144:9216]))
        final_insts.append(nc.scalar.dma_start(xT2[:, 3072:6144],
                                               xT2_d.ap()[:, 3072:6144]))
        final_insts.append(nc.scalar.dma_start(xT2[:, 9216:12288],
                                               xT2_d.ap()[:, 9216:12288]))
        for half in range(2):
            final_insts.append(nc.scalar.dma_start(
                hstall[:, half, 0:376],
                hinit_d.ap()[:, 376 * half:376 * half + 376]))
            final_insts.append(nc.scalar.dma_start(
                cini[:, half, 0:376],
                cinit_d.ap()[:, 376 * half:376 * half + 376]))
        final_insts.append(nc.scalar.dma_start(wpk, wpack_d.ap()))

        wi = {'f': wpk[:, 0:512], 'b': wpk[:, 1024:1536]}
        wh = {'f': wpk[:, 512:1024], 'b': wpk[:, 1536:2048]}
        owf = wpk[:, 2048:2816]
        owb = wpk[:, 2816:3584]
        bias = {'f': vpk[:, 0:4], 'b': vpk[:, 4:8]}      # per-gate biases
        ob = vpk[:, 8:14]
        ipb = vpk[:, 14:15]

        X = big.tile([HC, NCOLX], BF16, tag="X")
        Xr = big.tile([HC, NCOLX], BF16, tag="Xr")
        un = big.tile([HC, 2 * BS * 1024], BF16, tag="un")

        # ---- per-engine pre-observers: consume DMA semaphores via tiny
        # single-wait garbage ops so real instructions keep <=1 sync wait.
        # The xT2 slice observers are emitted just before the first tile
        # that needs each slice (inside the in_proj loop below).
        trash_a = const.tile([HC, 4], F32)
        trash_d = const.tile([HC, 4], F32)
        nc.tensor.ldweights(ipw[:, 0:1])                 # ipw DMA
        nc.tensor.ldweights(brow[:, 0:1])                # brow DMA
        nc.scalar.activation(trash_a[:, 0:1], vpk[:, 0:1], AF.Copy)   # vpack
        nc.vector.tensor_copy(trash_d[:, 0:1], vpk[:, 0:1])           # vpack
        tc.no_sync_barrier()

        # ---------------- phase 1: in_proj -> X / Xr ----------------
        # 24 psum chunks of 512 (12 per buffer); tile q covers chunks
        # 4q..4q+3; X tiles evicted by ACT, Xr tiles by DVE, interleaved so
        # both engines run while the PE streams ahead.
        xobs_cell = {0: xT2[:, 2047:2048], 1: xT2[:, 8191:8192],
                     2: xT2[:, 4095:4096], 3: xT2[:, 10239:10240]}
        with nc.named_scope("in_proj"):
            last_x_op = last_xr_op = None
            ev_cell = {}                   # half -> last evict dst cell
            for q in range(6):
                half = q % 2               # 0 -> X (ACT), 1 -> Xr (DVE)
                ti = q // 2                # tile index within the buffer
                P = psum.tile([HC, 4, 512], F32, tag='P' + 'fb'[half],
                              name=f"pin{q}")
                deps0 = []
                if q in xobs_cell:
                    deps0.append(nc.tensor.ldweights(xobs_cell[q]))
                if ti >= 1:
                    # PE pre-observes the evict tick that freed this tag
                    deps0.append(nc.tensor.ldweights(ev_cell[half]))
                for cb in range(4):
                    ci = half * 12 + ti * 4 + cb
                    mmi = nc.tensor.matmul(P[:, cb, :], ipw,
                                           xT2[:, 512 * ci:512 * ci + 512],
                                           start=True, stop=True)
                    if cb == 0:
                        for dep in deps0:
                            add_dep_helper(mmi.ins, dep.ins, sync=False,
                                           reason="pre-observed")
                # single 2048-col eviction per tile (32 diag blocks)
                dst = _ap(X if half == 0 else Xr, 128 * 32 * ti,
                          [[128, 32], [1, 64]])
                src = P[:, :, :].rearrange("p a b -> p (a b)")
                if half == 0:
                    last_x_op = nc.scalar.activation(
                        dst, src, AF.Identity, bias=ipb)
                else:
                    last_xr_op = nc.vector.tensor_scalar_add(dst, src, ipb)
                ev_cell[half] = (X if half == 0 else Xr)[
                    :, 4096 * ti:4096 * ti + 1]
            # late consts: nosync-ordered after the last in_proj mm so the
            # scheduler doesn't let their DMA waits block the mm pipeline.
            for cell in (wpk[:, 0:1], hstall[:, 0, 0:1], hstall[:, 1, 0:1]):
                ldc = nc.tensor.ldweights(cell)
                add_dep_helper(ldc.ins, mmi.ins, sync=False,
                               reason="order after in_proj mms")
            # C's sole writer chain starts with this DVE copy of the cinit
            # DMA, so scan cell ops keep single-sem waits (no DMA dep on C).
            # tch carries the DVE self-wait; the s=0 cm ops take a nosync
            # edge on it (V2's "c touch first" idiom). Emitted after the Xr
            # evictions so the copy's DMA wait can't delay them.
            for hh in range(2):        # one copy per cinit DMA: 1 wait each
                ccp = nc.vector.tensor_copy(C[:, hh, :], cini[:, hh, :])
                add_dep_helper(ccp.ins, last_xr_op.ins, sync=False,
                               reason="order after in_proj evictions")
            # col 376+ is copy-written but never cm-written: no WAR back-edge
            tch = nc.vector.tensor_copy(trash_d[:, 1:2], C[:, 1, 376:377])

        a_g, b_g, L_g, jf_g, jb_g = _geom()

        # ---------------- phase 2: scan (+ fused un scatter) ------------
        Xbuf = {'f': X, 'b': Xr}
        with nc.named_scope("scan"):
            # PE pre-observes eviction completion (frees pin psum tags AND
            # guarantees X/Xr contents) via two garbage ldweights.
            # cells written by the LAST eviction op (tile ti=2, col 8192)
            ldx = nc.tensor.ldweights(X[:, 8192:8193])        # ACT tick
            ldxr = nc.tensor.ldweights(Xr[:, 8192:8193])      # DVE tick
            first_mm_deps = [ldx, ldxr]
            sfio_prev = {}
            tgt_prev = {}
            hst_prev = {'f': hstall[:, 0, :], 'b': hstall[:, 1, :]}
            tc_prev = {}
            GORDER = (1, 0, 3, 2)          # ACT consumer order: f, i, g, o
            for s in range(S):
                nd = 94 - 4 * s
                N = BS * nd
                step = {}
                poff = 0
                for di, dr in enumerate('fb'):
                    P = psum.tile([HC, 4, 512], F32, tag='P' + dr,
                                  name=f"P{dr}{s}")
                    off = 257 * s if dr == 'f' else 255 * s + 15
                    xap = _ap(Xbuf[dr], off, [[128, nd], [16, BS]])
                    h0 = 0 if s == 0 else 8
                    hap = hst_prev[dr][:, h0:h0 + N]
                    # Wi mms in ACT-consumer gate order, each pre-observing
                    # only ITS gate's s-1 ACT read: the PE restarts as soon
                    # as sigma_f(s-1) fires instead of waiting sigma_o.
                    # ONE observer ld per dir-step reading the tgt cell
                    # of s-1 (the LAST ACT op this dir's mms depend on):
                    # it absorbs the mm's whole ACT dep class, leaving each
                    # mm its single group-serialization PE self-wait.
                    if s >= 1:
                        ldt = nc.tensor.ldweights(tgt_prev[dr][0:1, 0:1])
                    for g in GORDER:
                        if s >= 1:
                            deps0 = [ldt]
                        else:
                            deps0 = first_mm_deps if g == 1 else []
                        mmi = nc.tensor.matmul(
                            P[:, g, poff:poff + N],
                            wi[dr][:, g * HC:(g + 1) * HC],
                            xap, start=True, stop=False)
                        for dep in deps0:
                            add_dep_helper(mmi.ins, dep.ins, sync=False,
                                           reason="pre-observed")
                    # K=1 bias prefill ACCUMULATES mid-group (start=False)
                    # so the merged sigmoid needs no per-gate ACT bias
                    ones_ap = brow[:, 768:768 + N]
                    for g in (1, 0, 2):
                        nc.tensor.matmul(
                            P[:, g, 0:N],
                            brow[:, di * 384 + g * HC:di * 384 + (g + 1) * HC],
                            ones_ap, start=False, stop=False)
                    for g in GORDER:
                        nc.tensor.matmul(P[:, g, poff:poff + N],
                                         wh[dr][:, g * HC:(g + 1) * HC],
                                         hap, start=False, stop=True)
                    sfio = etmp.tile([HC, 3, 384], BF16, tag="sfio",
                                     name=f"sfio{dr}{s}")
                    tgt = etmp.tile([HC, 384], BF16, tag="tg",
                                    name=f"tg{dr}{s}")
                    sfio_prev[dr] = sfio
                    tgt_prev[dr] = tgt
                    step[dr] = (sfio, tgt, P)
                # ---- ACT gate ops, dir-INTERLEAVED in consumer order so
                # the b-dir's sigma_f fires early (frees the PE's b psum
                # banks ~1.2us sooner) while the f chain loses nothing ----
                for di, dr in enumerate('fb'):
                    sfio, tgt, P = step[dr]
                    nc.scalar.activation(sfio[:, 0:3, 0:N], P[:, 0:3, 0:N],
                                         AF.Sigmoid)
                for di, dr in enumerate('fb'):
                    sfio, tgt, P = step[dr]
                    nc.scalar.activation(tgt[:, 0:N], P[:, 3, 0:N], AF.Tanh,
                                         bias=bias[dr][:, 3:4])
                # ---- cell update: c = sf*c + si*tg (c preloaded by the
                # cinit DMA at s=0, shifted window afterwards); pm goes
                # to its own tile so tanh-g stays tgt's only writer.
                # Dir-interleaved to match the ACT emission. ----
                cells = {}
                for di, dr in enumerate('fb'):
                    sfio, tgt, P = step[dr]
                    c_sl = C[:, di, 8 * s:8 * s + N]
                    cells[dr] = c_sl
                    cm = nc.vector.tensor_mul(c_sl, c_sl, sfio[:, 1, 0:N])
                    if s == 0 and dr == 'f':
                        add_dep_helper(cm.ins, tch.ins, sync=False,
                                       reason="c touch first")
                for di, dr in enumerate('fb'):
                    sfio, tgt, P = step[dr]
                    pm = etmp.tile([HC, 384], BF16, tag="pm",
                                   name=f"pm{dr}{s}")
                    nc.vector.tensor_mul(pm[:, 0:N], sfio[:, 0, 0:N],
                                         tgt[:, 0:N])
                    nc.vector.tensor_add(cells[dr], cells[dr], pm[:, 0:N])
                for di, dr in enumerate('fb'):
                    sfio, tgt, P = step[dr]
                    c_sl = cells[dr]
                    tct = etmp.tile([HC, 384], BF16, tag="tc",
                                    name=f"tc{dr}{s}")
                    nc.scalar.activation(tct[:, 0:N], c_sl, AF.Tanh)
                    tc_prev[dr] = tct
                    hst = hstall[:, 2 + 2 * s + di, :]
                    nc.vector.tensor_mul(hst[:, 0:N], sfio[:, 2, 0:N],
                                         tct[:, 0:N])
                    hst_prev[dr] = hst
                    # scatter this step's h into un on the idle GPSIMD:
                    # pieces = (src_off, src_stride, cnt, dst_off, dst_stride)
                    if dr == 'f':
                        pieces = [(0, 4, 32 - 2 * s, 32 * s, 1),
                                  (4 * (32 - 2 * s), 8, 31 - s,
                                   30 * s + 62, 32),
                                  (4 * (33 - 2 * s), 8, 31 - s,
                                   30 * s + 63, 32)]
                    else:
                        pieces = [(0, 8, 32 - s, 4096 + 2 * s, 32),
                                  (4, 8, 31 - s, 4096 + 2 * s + 1, 32),
                                  (4 * (63 - 2 * s), 4, 31 - 2 * s,
                                   4096 + 993 - 30 * s, 1)]
                    for soff, sstr, cnt, doff, dstr in pieces:
                        hsrc = bass.AP(
                            tensor=hst.tensor, offset=hst.offset + soff,
                            ap=[hst.ap[0]] + [[1, BS], [sstr, cnt]])
                        last_un = nc.gpsimd.tensor_copy(
                            _ap(un, doff, [[1024, BS], [dstr, cnt]]), hsrc)

        # ---------------- phase 3: output projection ----------------
        # m-th output chunk evicts (with bias) to a contiguous 4096-col
        # slice of dead X (ACT, even m) or dead Xr (DVE, odd m), then DMAs
        # out as 128 contiguous 8KB descriptors.
        with nc.named_scope("out_proj"):
            ldun = nc.tensor.ldweights(un[:, 4639:4640])            # POOL tick
            ldtg = nc.tensor.ldweights(tc_prev['b'][:, 0:1])        # ACT tick
            evA = big.tile([HC, NCOLX], BF16, tag="X", name="evA")
            evB = big.tile([HC, NCOLX], BF16, tag="Xr", name="evB")

            last_ev = {0: None, 1: None}   # per-tag last evict dst
            for m in range(6):
                ev = evA if m % 2 == 0 else evB
                ow_m_f = owf[:, m * HC:(m + 1) * HC]
                ow_m_b = owb[:, m * HC:(m + 1) * HC]
                for half in range(2):
                    P = psum.tile([HC, 4, 512], F32, tag='P' + 'fb'[half],
                                  name=f"Po{m}{half}")
                    deps = []
                    if m == 0 and half == 0:
                        deps = [ldun, ldtg]
                    elif m == 0 and half == 1:
                        deps = [nc.tensor.ldweights(tc_prev['b'][:, 0:1])]
                    elif last_ev[half] is not None:
                        # PE pre-observes the evict tick that freed this tag
                        deps.append(nc.tensor.ldweights(last_ev[half]))
                        last_ev[half] = None
                    for cb in (1, 0, 2, 3):
                        ch = half * 4 + cb
                        # bwd-half mm FIRST: its un region contains the
                        # latest-produced cells, so its Pool need matches
                        # the ldun observer's count and absorbs cleanly
                        mmi = nc.tensor.matmul(
                            P[:, cb, :], ow_m_b,
                            un[:, 4096 + ch * 512:4096 + (ch + 1) * 512],
                            start=True, stop=False)
                        if cb == 1:
                            for dep in deps:
                                add_dep_helper(mmi.ins, dep.ins, sync=False,
                                               reason="pre-observed")
                        last_mm = nc.tensor.matmul(
                            P[:, cb, :], ow_m_f,
                            un[:, ch * 512:(ch + 1) * 512],
                            start=False, stop=True)
                    # single 2048-col eviction per psum tile
                    dst = ev[:, (m // 2) * 4096 + half * 2048:
                             (m // 2) * 4096 + half * 2048 + 2048]
                    src = P[:, :, :].rearrange("p a b -> p (a b)")
                    if m % 2 == 0:
                        last_act_evi = nc.scalar.activation(
                            dst, src, AF.Identity, bias=ob[:, m:m + 1])
                    else:
                        last_dve_evi = nc.vector.tensor_scalar_add(
                            dst, src, ob[:, m:m + 1])
                    last_ev[half] = dst[:, 0:1]
                # per-m DMA: src/dst contiguous per partition (8KB descs)
                final_insts.append(nc.gpsimd.dma_start(
                    out_d.ap()[:, m, :],
                    ev[:, (m // 2) * 4096:(m // 2) * 4096 + 4096]))
            final_insts += [last_mm, last_act_evi, last_dve_evi, last_un]
            for fi in final_insts:
                nop = nc.sync.nop()
                add_dep_helper(nop.ins, fi.ins, sync=True,
                               reason="drain diet: pre-observe final ticks")
    return nc


def _lstm_pad_states(Wh, b):
    """State after j pad steps (x=0): gates = b + Wh@h. Returns (17,HC) x2."""
    h = np.zeros(HC, np.float32)
    c = np.zeros(HC, np.float32)
    hs, cs = [h], [c]
    for _ in range(S):
        g = b + h @ Wh.T
        i, f, o, gg = g[0:HC], g[HC:2 * HC], g[2 * HC:3 * HC], g[3 * HC:]
        sig = lambda z: 1.0 / (1.0 + np.exp(-z))
        c = sig(f) * c + sig(i) * np.tanh(gg)
        h = sig(o) * np.tanh(c)
        hs.append(h.astype(np.float32))
        cs.append(c.astype(np.float32))
    return np.stack(hs), np.stack(cs)


def _pack_indices():
    """Host gather indices: xT2 col -> (bi, r, w) for live slots."""
    a, b, L, jf, jb = _geom()
    cols_f, cols_b = [], []
    src_b, src_r, src_w = [], [], []
    for d in range(D):
        for bi in range(BS):
            for s in range(int(L[d])):
                r = int(a[d]) + s
                w = d - 2 * r
                cols_f.append(64 * d + 16 * bi + s)
                cols_b.append(64 * d + 16 * bi + 16 - int(L[d]) + s)
                src_b.append(bi)
                src_r.append(r)
                src_w.append(w)
    return (np.array(cols_f), np.array(cols_b), np.array(src_b),
            np.array(src_r), np.array(src_w))


_PACK = _pack_indices()


def _prep_inputs(inputs):
    """Host-side weight reshaping + pixel packing -> per-core in_maps."""
    import ml_dtypes
    bf = ml_dtypes.bfloat16

    def cast(a):
        return np.ascontiguousarray(a, np.float32).astype(bf)

    x = np.asarray(inputs['x'], np.float32)
    fwd_Wh = np.asarray(inputs['fwd_Wh'], np.float32)
    bwd_Wh = np.asarray(inputs['bwd_Wh'], np.float32)
    fwd_b = np.asarray(inputs['fwd_b'], np.float32)
    bwd_b = np.asarray(inputs['bwd_b'], np.float32)
    wpack = np.concatenate([
        np.asarray(inputs['fwd_Wi'], np.float32).T, fwd_Wh.T,
        np.asarray(inputs['bwd_Wi'], np.float32).T, bwd_Wh.T,
        np.asarray(inputs['out_w'], np.float32)[:, :HC].T,
        np.asarray(inputs['out_w'], np.float32)[:, HC:].T,
    ], axis=1)                                             # (128, 3584)
    vpack = np.concatenate([
        fwd_b.reshape(4, HC).T, bwd_b.reshape(4, HC).T,
        np.asarray(inputs['out_b'], np.float32).reshape(6, HC).T,
        np.asarray(inputs['in_proj_b'], np.float32).reshape(HC, 1),
    ], axis=1)                                             # (128, 15)

    # per-diagonal init states from the pad-state tables
    a, b, L, jf, jb = _geom()
    hinit = np.zeros((HC, 2 * 376), np.float32)
    cinit = np.zeros((HC, 2 * 376), np.float32)
    for di, (Wh_, b_) in enumerate(((fwd_Wh, fwd_b), (bwd_Wh, bwd_b))):
        hs, cs = _lstm_pad_states(Wh_, b_)
        j = jf if di == 0 else jb
        cols = 376 * di + 4 * np.arange(D)[:, None] + np.arange(BS)[None, :]
        hinit[:, cols.reshape(-1)] = np.repeat(hs[j], BS, axis=0).T
        cinit[:, cols.reshape(-1)] = np.repeat(cs[j], BS, axis=0).T

    brow = np.concatenate([fwd_b[0:384], bwd_b[0:384],
                           np.ones(384, np.float32)]).reshape(1, 1152)

    common = {
        "brow": cast(brow),
        "in_projT": cast(np.asarray(inputs['in_proj_w'], np.float32).T
                         / 255.0),
        "wpack": cast(wpack),
        "vpack": np.ascontiguousarray(vpack),
        "hinit": cast(hinit),
        "cinit": cast(cinit),
    }
    cols_f, cols_b, sb, sr, sw = _PACK
    in_maps = []
    for c in range(NCORES):
        xs = x[c * BS:(c + 1) * BS]                        # (4, 3, 32, 32)
        vals = xs[sb, :, sr, sw].T                         # (3, nlive)
        xT2c = np.zeros((3, 2 * 6144), np.float32)
        xT2c[:, cols_f] = vals
        xT2c[:, 6144 + cols_b] = vals
        in_maps.append({"xT2": cast(xT2c), **common})
    return in_maps


def _assemble(results):
    outs = []
    for r in results:
        lg = np.asarray(r["out"], dtype=np.float32)        # (128, 6, 4096)
        lg = lg.transpose(1, 0, 2).reshape(6, HC, BS, H, W)
        lg = lg.transpose(2, 0, 1, 3, 4)
        outs.append(lg.reshape(BS, 768, H, W))
    full = np.concatenate(outs, axis=0)
    return np.ascontiguousarray(
        full.reshape(32, 3, 256, H, W).astype(np.float32))


def kernel(**inputs):
    nc = bass.Bass("TRN2", target_bir_lowering=False, debug=False)
    build(nc)
    in_maps = _prep_inputs(inputs)
    res = run_bass_kernel_spmd(nc, in_maps, core_ids=list(range(NCORES)))
    return _assemble(res.results)


if __name__ == "__main__":
    nc = bass.Bass("TRN2", target_bir_lowering=False, debug=False)
    build(nc)
    print("IR build OK")


# revision 81
# speedup vs baseline: 1.0596x; 1.0596x over previous
"""Trainium2 Bass kernel V3 for the PixelRNN Diagonal BiLSTM problem.

Contract: kernel(**inputs) takes FULL unsharded inputs and returns the FULL
(32, 3, 256, 32, 32) float32 output. Pure data-parallel over 8 NeuronCores
(4 images each), weights replicated, no collectives.

V3 vs V2 (see sim_v3.py for geometry validation):
  * Halved scan: only ~half the skewed-grid cells are real pixels. Prefix
    pad-cell LSTM states are input-independent (x=0 there), so the host
    precomputes the 17 possible pad states from the weights and each
    diagonal's scan starts at its first real cell with a per-diagonal init
    (h_init, c_init) table. Real segments are <= 16 steps, so the scan is
    16 macro-steps/dir over 4096 columns (vs 32 steps over 8064).
  * No scatter phase: the host packs the RAW pixels in scan-slot order
    (xT2 col = 64d + 16bi + s, bwd copy right-aligned within each 16-slot
    block), so the in_proj psum evictions write X/Xr directly with
    dst-contiguous 64-col runs. X layout col = 128d + 16bi + s leaves a
    dead upper half per diagonal; windows are offset 257s (fwd) and
    255s+15 (bwd) with strides [[128, nd], [16, 4]], nd = 94-4s.
  * un-scatters moved to the idle GPSIMD engine (3 pieces per dir-step).
  * out_proj: evictions go to per-m contiguous 4096-col slices (ACT for
    even m on dead X, DVE for odd m on dead Xr) and each m DMAs out as
    128 contiguous 8KB descriptors right after its eviction (the V2
    layout produced 4KB descriptors and a ~43 us DMA tail).
  * Cell state C is bf16 (DVE 2x perf mode; rel err 5.0e-3 vs 4.6e-3).
  * Scan matmuls run in the ACT consumer gate order (f,i,g,o) with
    per-gate psum-free pre-observers; ACT gate ops are dir-interleaved.
  * Input DMAs split across the SP and ACT HWDGE channels in need order;
    per-slice xT2 observers sit just before the first tile needing them.

Steady-state (HW, 8 cores): ~167 us vs the 285 us V2 baseline. The scan
(~106 us) is bound by the ACT engine's 10 transcendental ops per step
(~450 ns each incl. 185 ns fixed access cost); late steps reach 100% ACT
busy with the PE at the full 2.4 GHz p-state.

Per-core SBUF: X/Xr [128, 12288] bf16, hstall [128, 34, 384] bf16,
C [128, 2, 384] bf16, un [128, 8192] bf16, xT2 [3, 12288] bf16,
weights [128, 3584] bf16.
"""
from contextlib import ExitStack

import numpy as np

import concourse.bass as bass
import concourse.tile as tile
from concourse.tile import add_dep_helper
from concourse import mybir
from concourse.bass_utils import run_bass_kernel_spmd

AF = mybir.ActivationFunctionType
F32 = mybir.dt.float32
BF16 = mybir.dt.bfloat16
U32 = mybir.dt.uint32

BS = 4            # batch shard per core
NCORES = 8
H = W = 32
HC = 128
D = 94            # anti-diagonals of the skewed grid
S = 16            # max real-segment length
NCOLX = 96 * 128  # X/Xr alloc: 94 d-blocks (live) padded to 96


def _geom():
    d = np.arange(D)
    r0 = np.maximum(0, d - 62)
    r1 = np.minimum(31, d)
    a = np.maximum(r0, -(-(d - 31) // 2))
    b = np.minimum(r1, d // 2)
    L = b - a + 1
    return a, b, L, a - r0, r1 - b


def _ap(t, off, dims):
    """Free-dim-strided AP on SBUF tile t: dims = [[stride, count], ...]."""
    a = t[:, :]
    return bass.AP(tensor=a.tensor, offset=a.offset + off, ap=[a.ap[0]] + dims)


def build(nc):
    # ---------------- DRAM I/O ----------------
    xT2_d = nc.dram_tensor("xT2", (3, 2 * 6144), BF16, kind="ExternalInput")
    ipw_d = nc.dram_tensor("in_projT", (3, HC), BF16, kind="ExternalInput")
    wpack_d = nc.dram_tensor("wpack", (HC, 3584), BF16, kind="ExternalInput")
    vpack_d = nc.dram_tensor("vpack", (HC, 15), F32, kind="ExternalInput")
    hinit_d = nc.dram_tensor("hinit", (HC, 2 * 376), BF16, kind="ExternalInput")
    cinit_d = nc.dram_tensor("cinit", (HC, 2 * 376), BF16,
                             kind="ExternalInput")
    out_d = nc.dram_tensor("out", (HC, 6, BS * 1024), BF16,
                           kind="ExternalOutput")

    with tile.TileContext(nc) as tc, ExitStack() as ctx:
        const = ctx.enter_context(tc.tile_pool(name="const", bufs=1))
        big = ctx.enter_context(tc.tile_pool(name="big", bufs=1))
        etmp = ctx.enter_context(tc.tile_pool(name="etmp", bufs=4))
        psum = ctx.enter_context(tc.tile_pool(name="psum", bufs=1,
                                              space="PSUM"))

        # DMA plan: each issuer's HWDGE channel moves its transfers serially,
        # so the SP carries the in_proj-critical path (ipw, xa1, vpk, xb1 in
        # need order) while ACT carries the rest in need order.
        final_insts = []
        ipw = const.tile([3, HC], BF16)
        xT2 = const.tile([3, 2 * 6144], BF16)
        vpk = const.tile([HC, 15], F32)
        wpk = const.tile([HC, 3584], BF16)
        cini = const.tile([HC, 2, 384], BF16, name="cini")
        C = const.tile([HC, 2, 384], BF16, name="C")
        # hstall: slice 0/1 = fwd/bwd init h; slice 2+2s+di = step-(s,dir) h
        hstall = const.tile([HC, 34, 384], BF16, name="hstall")

        final_insts.append(nc.sync.dma_start(ipw, ipw_d.ap()))
        final_insts.append(nc.sync.dma_start(xT2[:, 0:3072],
                                             xT2_d.ap()[:, 0:3072]))
        final_insts.append(nc.sync.dma_start(vpk, vpack_d.ap()))
        final_insts.append(nc.sync.dma_start(xT2[:, 6144:9216],
                                             xT2_d.ap()[:, 6144:9216]))
        final_insts.append(nc.scalar.dma_start(xT2[:, 3072:6144],
                                               xT2_d.ap()[:, 3072:6144]))
        final_insts.append(nc.scalar.dma_start(xT2[:, 9216:12288],
                                               xT2_d.ap()[:, 9216:12288]))
        for half in range(2):
            final_insts.append(nc.scalar.dma_start(
                hstall[:, half, 0:376],
                hinit_d.ap()[:, 376 * half:376 * half + 376]))
            final_insts.append(nc.scalar.dma_start(
                cini[:, half, 0:376],
                cinit_d.ap()[:, 376 * half:376 * half + 376]))
        final_insts.append(nc.scalar.dma_start(wpk, wpack_d.ap()))

        wi = {'f': wpk[:, 0:512], 'b': wpk[:, 1024:1536]}
        wh = {'f': wpk[:, 512:1024], 'b': wpk[:, 1536:2048]}
        owf = wpk[:, 2048:2816]
        owb = wpk[:, 2816:3584]
        bias = {'f': vpk[:, 0:4], 'b': vpk[:, 4:8]}      # per-gate biases
        ob = vpk[:, 8:14]
        ipb = vpk[:, 14:15]

        X = big.tile([HC, NCOLX], BF16, tag="X")
        Xr = big.tile([HC, NCOLX], BF16, tag="Xr")
        un = big.tile([HC, 2 * BS * 1024], BF16, tag="un")

        # ---- per-engine pre-observers: consume DMA semaphores via tiny
        # single-wait garbage ops so real instructions keep <=1 sync wait.
        # The xT2 slice observers are emitted just before the first tile
        # that needs each slice (inside the in_proj loop below).
        trash_a = const.tile([HC, 4], F32)
        trash_d = const.tile([HC, 4], F32)
        nc.tensor.ldweights(ipw[:, 0:1])                 # ipw DMA
        nc.scalar.activation(trash_a[:, 0:1], vpk[:, 0:1], AF.Copy)   # vpack
        nc.vector.tensor_copy(trash_d[:, 0:1], vpk[:, 0:1])           # vpack
        tc.no_sync_barrier()

        # ---------------- phase 1: in_proj -> X / Xr ----------------
        # 24 psum chunks of 512 (12 per buffer); tile q covers chunks
        # 4q..4q+3; X tiles evicted by ACT, Xr tiles by DVE, interleaved so
        # both engines run while the PE streams ahead.
        xobs_cell = {0: xT2[:, 2047:2048], 1: xT2[:, 8191:8192],
                     2: xT2[:, 4095:4096], 3: xT2[:, 10239:10240]}
        with nc.named_scope("in_proj"):
            last_x_op = last_xr_op = None
            ev_cell = {}                   # half -> last evict dst cell
            for q in range(6):
                half = q % 2               # 0 -> X (ACT), 1 -> Xr (DVE)
                ti = q // 2                # tile index within the buffer
                P = psum.tile([HC, 4, 512], F32, tag='P' + 'fb'[half],
                              name=f"pin{q}")
                deps0 = []
                if q in xobs_cell:
                    deps0.append(nc.tensor.ldweights(xobs_cell[q]))
                if ti >= 1:
                    # PE pre-observes the evict tick that freed this tag
                    deps0.append(nc.tensor.ldweights(ev_cell[half]))
                for cb in range(4):
                    ci = half * 12 + ti * 4 + cb
                    mmi = nc.tensor.matmul(P[:, cb, :], ipw,
                                           xT2[:, 512 * ci:512 * ci + 512],
                                           start=True, stop=True)
                    if cb == 0:
                        for dep in deps0:
                            add_dep_helper(mmi.ins, dep.ins, sync=False,
                                           reason="pre-observed")
                # single 2048-col eviction per tile (32 diag blocks)
                dst = _ap(X if half == 0 else Xr, 128 * 32 * ti,
                          [[128, 32], [1, 64]])
                src = P[:, :, :].rearrange("p a b -> p (a b)")
                if half == 0:
                    last_x_op = nc.scalar.activation(
                        dst, src, AF.Identity, bias=ipb)
                else:
                    last_xr_op = nc.vector.tensor_scalar_add(dst, src, ipb)
                ev_cell[half] = (X if half == 0 else Xr)[
                    :, 4096 * ti:4096 * ti + 1]
            # late consts: nosync-ordered after the last in_proj mm so the
            # scheduler doesn't let their DMA waits block the mm pipeline.
            for cell in (wpk[:, 0:1], hstall[:, 0, 0:1], hstall[:, 1, 0:1]):
                ldc = nc.tensor.ldweights(cell)
                add_dep_helper(ldc.ins, mmi.ins, sync=False,
                               reason="order after in_proj mms")
            # C's sole writer chain starts with this DVE copy of the cinit
            # DMA, so scan cell ops keep single-sem waits (no DMA dep on C).
            # tch carries the DVE self-wait; the s=0 cm ops take a nosync
            # edge on it (V2's "c touch first" idiom). Emitted after the Xr
            # evictions so the copy's DMA wait can't delay them.
            for hh in range(2):        # one copy per cinit DMA: 1 wait each
                ccp = nc.vector.tensor_copy(C[:, hh, :], cini[:, hh, :])
                add_dep_helper(ccp.ins, last_xr_op.ins, sync=False,
                               reason="order after in_proj evictions")
            # col 376+ is copy-written but never cm-written: no WAR back-edge
            tch = nc.vector.tensor_copy(trash_d[:, 1:2], C[:, 1, 376:377])

        a_g, b_g, L_g, jf_g, jb_g = _geom()

        # ---------------- phase 2: scan (+ fused un scatter) ------------
        Xbuf = {'f': X, 'b': Xr}
        with nc.named_scope("scan"):
            # PE pre-observes eviction completion (frees pin psum tags AND
            # guarantees X/Xr contents) via two garbage ldweights.
            # cells written by the LAST eviction op (tile ti=2, col 8192)
            ldx = nc.tensor.ldweights(X[:, 8192:8193])        # ACT tick
            ldxr = nc.tensor.ldweights(Xr[:, 8192:8193])      # DVE tick
            first_mm_deps = [ldx, ldxr]
            sfio_prev = {}
            tgt_prev = {}
            hst_prev = {'f': hstall[:, 0, :], 'b': hstall[:, 1, :]}
            tc_prev = {}
            GORDER = (1, 0, 3, 2)          # ACT consumer order: f, i, g, o
            for s in range(S):
                nd = 94 - 4 * s
                N = BS * nd
                step = {}
                poff = 0
                for di, dr in enumerate('fb'):
                    P = psum.tile([HC, 4, 512], F32, tag='P' + dr,
                                  name=f"P{dr}{s}")
                    off = 257 * s if dr == 'f' else 255 * s + 15
                    xap = _ap(Xbuf[dr], off, [[128, nd], [16, BS]])
                    h0 = 0 if s == 0 else 8
                    hap = hst_prev[dr][:, h0:h0 + N]
                    # Wi mms in ACT-consumer gate order, each pre-observing
                    # only ITS gate's s-1 ACT read: the PE restarts as soon
                    # as sigma_f(s-1) fires instead of waiting sigma_o.
                    for g in GORDER:
                        if s >= 1:
                            cell = (tgt_prev[dr][0:1, 0:1] if g == 3 else
                                    sfio_prev[dr][0:1, g, 0:1])
                            deps0 = [nc.tensor.ldweights(cell)]
                        else:
                            deps0 = first_mm_deps if g == 1 else []
                        mmi = nc.tensor.matmul(
                            P[:, g, poff:poff + N],
                            wi[dr][:, g * HC:(g + 1) * HC],
                            xap, start=True, stop=False)
                        for dep in deps0:
                            add_dep_helper(mmi.ins, dep.ins, sync=False,
                                           reason="pre-observed")
                    for g in GORDER:
                        nc.tensor.matmul(P[:, g, poff:poff + N],
                                         wh[dr][:, g * HC:(g + 1) * HC],
                                         hap, start=False, stop=True)
                    sfio = etmp.tile([HC, 3, 384], BF16, tag="sfio",
                                     name=f"sfio{dr}{s}")
                    tgt = etmp.tile([HC, 384], BF16, tag="tg",
                                    name=f"tg{dr}{s}")
                    sfio_prev[dr] = sfio
                    tgt_prev[dr] = tgt
                    step[dr] = (sfio, tgt, P)
                # ---- ACT gate ops, dir-INTERLEAVED in consumer order so
                # the b-dir's sigma_f fires early (frees the PE's b psum
                # banks ~1.2us sooner) while the f chain loses nothing ----
                for g in GORDER:
                    for di, dr in enumerate('fb'):
                        sfio, tgt, P = step[dr]
                        psl = P[:, g, poff:poff + N]
                        if g == 3:
                            nc.scalar.activation(tgt[:, 0:N], psl, AF.Tanh,
                                                 bias=bias[dr][:, 3:4])
                        else:
                            nc.scalar.activation(sfio[:, g, 0:N], psl,
                                                 AF.Sigmoid,
                                                 bias=bias[dr][:, g:g + 1])
                # ---- cell update: c = sf*c + si*tg (c preloaded by the
                # cinit DMA at s=0, shifted window afterwards); pm goes
                # to its own tile so tanh-g stays tgt's only writer.
                # Dir-interleaved to match the ACT emission. ----
                cells = {}
                for di, dr in enumerate('fb'):
                    sfio, tgt, P = step[dr]
                    c_sl = C[:, di, 8 * s:8 * s + N]
                    cells[dr] = c_sl
                    cm = nc.vector.tensor_mul(c_sl, c_sl, sfio[:, 1, 0:N])
                    if s == 0 and dr == 'f':
                        add_dep_helper(cm.ins, tch.ins, sync=False,
                                       reason="c touch first")
                for di, dr in enumerate('fb'):
                    sfio, tgt, P = step[dr]
                    pm = etmp.tile([HC, 384], BF16, tag="pm",
                                   name=f"pm{dr}{s}")
                    nc.vector.tensor_mul(pm[:, 0:N], sfio[:, 0, 0:N],
                                         tgt[:, 0:N])
                    nc.vector.tensor_add(cells[dr], cells[dr], pm[:, 0:N])
                for di, dr in enumerate('fb'):
                    sfio, tgt, P = step[dr]
                    c_sl = cells[dr]
                    tct = etmp.tile([HC, 384], BF16, tag="tc",
                                    name=f"tc{dr}{s}")
                    nc.scalar.activation(tct[:, 0:N], c_sl, AF.Tanh)
                    tc_prev[dr] = tct
                    hst = hstall[:, 2 + 2 * s + di, :]
                    nc.vector.tensor_mul(hst[:, 0:N], sfio[:, 2, 0:N],
                                         tct[:, 0:N])
                    hst_prev[dr] = hst
                    # scatter this step's h into un on the idle GPSIMD:
                    # pieces = (src_off, src_stride, cnt, dst_off, dst_stride)
                    if dr == 'f':
                        pieces = [(0, 4, 32 - 2 * s, 32 * s, 1),
                                  (4 * (32 - 2 * s), 8, 31 - s,
                                   30 * s + 62, 32),
                                  (4 * (33 - 2 * s), 8, 31 - s,
                                   30 * s + 63, 32)]
                    else:
                        pieces = [(0, 8, 32 - s, 4096 + 2 * s, 32),
                                  (4, 8, 31 - s, 4096 + 2 * s + 1, 32),
                                  (4 * (63 - 2 * s), 4, 31 - 2 * s,
                                   4096 + 993 - 30 * s, 1)]
                    for soff, sstr, cnt, doff, dstr in pieces:
                        hsrc = bass.AP(
                            tensor=hst.tensor, offset=hst.offset + soff,
                            ap=[hst.ap[0]] + [[1, BS], [sstr, cnt]])
                        last_un = nc.gpsimd.tensor_copy(
                            _ap(un, doff, [[1024, BS], [dstr, cnt]]), hsrc)

        # ---------------- phase 3: output projection ----------------
        # m-th output chunk evicts (with bias) to a contiguous 4096-col
        # slice of dead X (ACT, even m) or dead Xr (DVE, odd m), then DMAs
        # out as 128 contiguous 8KB descriptors.
        with nc.named_scope("out_proj"):
            ldun = nc.tensor.ldweights(un[:, 4639:4640])            # POOL tick
            ldtg = nc.tensor.ldweights(tc_prev['b'][:, 0:1])        # ACT tick
            evA = big.tile([HC, NCOLX], BF16, tag="X", name="evA")
            evB = big.tile([HC, NCOLX], BF16, tag="Xr", name="evB")

            last_ev = {0: None, 1: None}   # per-tag last evict dst
            for m in range(6):
                ev = evA if m % 2 == 0 else evB
                ow_m_f = owf[:, m * HC:(m + 1) * HC]
                ow_m_b = owb[:, m * HC:(m + 1) * HC]
                for half in range(2):
                    P = psum.tile([HC, 4, 512], F32, tag='P' + 'fb'[half],
                                  name=f"Po{m}{half}")
                    deps = []
                    if m == 0 and half == 0:
                        deps = [ldun, ldtg]
                    elif m == 0 and half == 1:
                        deps = [nc.tensor.ldweights(tc_prev['b'][:, 0:1])]
                    elif last_ev[half] is not None:
                        # PE pre-observes the evict tick that freed this tag
                        deps.append(nc.tensor.ldweights(last_ev[half]))
                        last_ev[half] = None
                    for cb in range(4):
                        ch = half * 4 + cb
                        mmi = nc.tensor.matmul(
                            P[:, cb, :], ow_m_f,
                            un[:, ch * 512:(ch + 1) * 512],
                            start=True, stop=False)
                        if cb == 0:
                            for dep in deps:
                                add_dep_helper(mmi.ins, dep.ins, sync=False,
                                               reason="pre-observed")
                        last_mm = nc.tensor.matmul(
                            P[:, cb, :], ow_m_b,
                            un[:, 4096 + ch * 512:4096 + (ch + 1) * 512],
                            start=False, stop=True)
                    # single 2048-col eviction per psum tile
                    dst = ev[:, (m // 2) * 4096 + half * 2048:
                             (m // 2) * 4096 + half * 2048 + 2048]
                    src = P[:, :, :].rearrange("p a b -> p (a b)")
                    if m % 2 == 0:
                        last_act_evi = nc.scalar.activation(
                            dst, src, AF.Identity, bias=ob[:, m:m + 1])
                    else:
                        last_dve_evi = nc.vector.tensor_scalar_add(
                            dst, src, ob[:, m:m + 1])
                    last_ev[half] = dst[:, 0:1]
                # per-m DMA: src/dst contiguous per partition (8KB descs)
                final_insts.append(nc.gpsimd.dma_start(
                    out_d.ap()[:, m, :],
                    ev[:, (m // 2) * 4096:(m // 2) * 4096 + 4096]))
            final_insts += [last_mm, last_act_evi, last_dve_evi, last_un]
            for fi in final_insts:
                nop = nc.sync.nop()
                add_dep_helper(nop.ins, fi.ins, sync=True,
                               reason="drain diet: pre-observe final ticks")
    return nc


def _lstm_pad_states(Wh, b):
    """State after j pad steps (x=0): gates = b + Wh@h. Returns (17,HC) x2."""
    h = np.zeros(HC, np.float32)
    c = np.zeros(HC, np.float32)
    hs, cs = [h], [c]
    for _ in range(S):
        g = b + h @ Wh.T
        i, f, o, gg = g[0:HC], g[HC:2 * HC], g[2 * HC:3 * HC], g[3 * HC:]
        sig = lambda z: 1.0 / (1.0 + np.exp(-z))
        c = sig(f) * c + sig(i) * np.tanh(gg)
        h = sig(o) * np.tanh(c)
        hs.append(h.astype(np.float32))
        cs.append(c.astype(np.float32))
    return np.stack(hs), np.stack(cs)


def _pack_indices():
    """Host gather indices: xT2 col -> (bi, r, w) for live slots."""
    a, b, L, jf, jb = _geom()
    cols_f, cols_b = [], []
    src_b, src_r, src_w = [], [], []
    for d in range(D):
        for bi in range(BS):
            for s in range(int(L[d])):
                r = int(a[d]) + s
                w = d - 2 * r
                cols_f.append(64 * d + 16 * bi + s)
                cols_b.append(64 * d + 16 * bi + 16 - int(L[d]) + s)
                src_b.append(bi)
                src_r.append(r)
                src_w.append(w)
    return (np.array(cols_f), np.array(cols_b), np.array(src_b),
            np.array(src_r), np.array(src_w))


_PACK = _pack_indices()


def _prep_inputs(inputs):
    """Host-side weight reshaping + pixel packing -> per-core in_maps."""
    import ml_dtypes
    bf = ml_dtypes.bfloat16

    def cast(a):
        return np.ascontiguousarray(a, np.float32).astype(bf)

    x = np.asarray(inputs['x'], np.float32)
    fwd_Wh = np.asarray(inputs['fwd_Wh'], np.float32)
    bwd_Wh = np.asarray(inputs['bwd_Wh'], np.float32)
    fwd_b = np.asarray(inputs['fwd_b'], np.float32)
    bwd_b = np.asarray(inputs['bwd_b'], np.float32)
    wpack = np.concatenate([
        np.asarray(inputs['fwd_Wi'], np.float32).T, fwd_Wh.T,
        np.asarray(inputs['bwd_Wi'], np.float32).T, bwd_Wh.T,
        np.asarray(inputs['out_w'], np.float32)[:, :HC].T,
        np.asarray(inputs['out_w'], np.float32)[:, HC:].T,
    ], axis=1)                                             # (128, 3584)
    vpack = np.concatenate([
        fwd_b.reshape(4, HC).T, bwd_b.reshape(4, HC).T,
        np.asarray(inputs['out_b'], np.float32).reshape(6, HC).T,
        np.asarray(inputs['in_proj_b'], np.float32).reshape(HC, 1),
    ], axis=1)                                             # (128, 15)

    # per-diagonal init states from the pad-state tables
    a, b, L, jf, jb = _geom()
    hinit = np.zeros((HC, 2 * 376), np.float32)
    cinit = np.zeros((HC, 2 * 376), np.float32)
    for di, (Wh_, b_) in enumerate(((fwd_Wh, fwd_b), (bwd_Wh, bwd_b))):
        hs, cs = _lstm_pad_states(Wh_, b_)
        j = jf if di == 0 else jb
        cols = 376 * di + 4 * np.arange(D)[:, None] + np.arange(BS)[None, :]
        hinit[:, cols.reshape(-1)] = np.repeat(hs[j], BS, axis=0).T
        cinit[:, cols.reshape(-1)] = np.repeat(cs[j], BS, axis=0).T

    common = {
        "in_projT": cast(np.asarray(inputs['in_proj_w'], np.float32).T
                         / 255.0),
        "wpack": cast(wpack),
        "vpack": np.ascontiguousarray(vpack),
        "hinit": cast(hinit),
        "cinit": cast(cinit),
    }
    cols_f, cols_b, sb, sr, sw = _PACK
    in_maps = []
    for c in range(NCORES):
        xs = x[c * BS:(c + 1) * BS]                        # (4, 3, 32, 32)
        vals = xs[sb, :, sr, sw].T                         # (3, nlive)
        xT2c = np.zeros((3, 2 * 6144), np.float32)
        xT2c[:, cols_f] = vals
        xT2c[:, 6144 + cols_b] = vals
        in_maps.append({"xT2": cast(xT2c), **common})
    return in_maps


def _assemble(results):
    outs = []
    for r in results:
        lg = np.asarray(r["out"], dtype=np.float32)        # (128, 6, 4096)
        lg = lg.transpose(1, 0, 2).reshape(6, HC, BS, H, W)
        lg = lg.transpose(2, 0, 1, 3, 4)
        outs.append(lg.reshape(BS, 768, H, W))
    full = np.concatenate(outs, axis=0)
    return np.ascontiguousarray(
        full.reshape(32, 3, 256, H, W).astype(np.float32))


def kernel(**inputs):
    nc = bass.Bass("TRN2", target_bir_lowering=False, debug=False)
    build(nc)
    in_maps = _prep_inputs(inputs)
    res = run_bass_kernel_spmd(nc, in_maps, core_ids=list(range(NCORES)))
    return _assemble(res.results)


if __name__ == "__main__":
    nc = bass.Bass("TRN2", target_bir_lowering=False, debug=False)
    build(nc)
    print("IR build OK")
